# revision 1
# baseline (speedup 1.0000x reference)
"""Trainium2 Bass kernel for DecoupledMVRowSelfAttnProcessor (bs=6, seq=1024, C=1280,
20 heads, 6 views, row-wise MV attention). Self-contained: builds + compiles an 8-core
SPMD Tile kernel on first call, then runs it via run_bass_kernel_spmd.

Sharding:
  Phase A (token-sharded, 768 tok/core): X^T via PE-transpose, 9 QKV projections in
    bf16; K^T / V written to local DRAM and AllGathered (4-subgroup for base/ref
    batch halves, 8-group for MV); Q^T for base/ref stays SBUF-resident.
  Phase B (query-sharded base/ref, image-row-sharded MV): softmax over the kv
    partition axis (exp -> ones-column row sums -> reciprocal -> partition_broadcast)
    so no transposes are needed; O^T accumulates SBUF-resident. The MV out-projection
    runs here (all heads are core-local) and its token rows are AllGathered.
  Phase C (token-sharded): base/ref out-projections + combined bias + MV rows +
    residual, all accumulated in PSUM / DVE.
"""
import sys
sys.path.insert(0, '/opt/trn_rl_repo')
import contextlib
import math
import numpy as np

import concourse.bass as bass
import concourse.mybir as mybir
from concourse import bacc
from concourse.tile import TileContext
from concourse.bass_utils import run_bass_kernel_spmd
from concourse.masks import make_identity

f32 = mybir.dt.float32
bf16 = mybir.dt.bfloat16
u32 = mybir.dt.uint32
AF = mybir.ActivationFunctionType
ALU = mybir.AluOpType

NCORES = 8
BS, SEQ, C = 6, 1024, 1280
H, HD, NV = 20, 64, 6
IH = IW = 32
TOK = BS * SEQ            # 6144
TPC = TOK // NCORES       # 768
NCI = C // 128            # 10
QC = 256                  # query chunk (always batch-pure)
NQC = TPC // QC           # 3
KT = SEQ // 128           # 8
HALF = TOK // 2           # 3072
LKV = NV * IW             # 192
SCALE = 1.0 / math.sqrt(HD)

# plan tensor layout (uint32, per core):
#   [0:12]   krow[qc*4+j]  row base (rank*C) into K{b,r}G (4C, TPC)
#   [12:24]  kcol[qc*4+j]  col base into K{b,r}G
#   [24:27]  vrow[qc]      row base (btilde*SEQ) into V{b,r}G (HALF, C)
#   [27:51]  mrow[rl*6+v]  row base (rank*C) into {K,Q}mG (8C, TPC)
#   [51:75]  mcol[rl*6+v]  col base into {K,Q}mG
#   [75:79]  mvr[rl]       row base (r_gl*IW) within each SEQ block of VmG
#   [79:103] crow[tt*4+j]  row base into OmG (TOK, C)
PLAN_LEN = 128

_CACHE = {}


def _build():
    nc = bacc.Bacc("TRN2", target_bir_lowering=False, debug=False, num_devices=NCORES)

    hid = nc.declare_dram_parameter("hid_shard", [TPC, C], f32, isOutput=False)
    ref = nc.declare_dram_parameter("ref_shard", [TPC, C], f32, isOutput=False)
    WNAMES = ["Wq", "Wk", "Wv", "Wq_mv", "Wk_mv", "Wv_mv", "Wq_ref", "Wk_ref",
              "Wv_ref", "Wout", "Wout_mv", "Wout_ref"]
    Wn = {n: nc.declare_dram_parameter(n, [C, C], f32, isOutput=False) for n in WNAMES}
    bsum = nc.declare_dram_parameter("bsum", [1, C], f32, isOutput=False)
    plan = nc.declare_dram_parameter("plan", [1, PLAN_LEN], u32, isOutput=False)
    out = nc.declare_dram_parameter("out_shard", [TPC, C], f32, isOutput=True)

    with TileContext(nc) as tc, contextlib.ExitStack() as stack:
        const = stack.enter_context(tc.tile_pool(name="const", bufs=1))
        resid = stack.enter_context(tc.tile_pool(name="resident", bufs=1))
        dram = stack.enter_context(tc.tile_pool(name="dram", bufs=1, space="DRAM"))

        ident = const.tile([128, 128], bf16)
        make_identity(nc, ident[:])
        ones_row = const.tile([1, 128], bf16)   # lhsT for bias broadcast
        nc.any.memset(ones_row[:], 1.0)
        ones_col = const.tile([128, 1], bf16)   # lhsT for MV row sums
        nc.any.memset(ones_col[:], 1.0)
        bsum_bf = const.tile([1, C], bf16)
        nc.gpsimd.dma_start(bsum_bf[:], bsum[:])
        plan_sb = const.tile([1, PLAN_LEN], u32)
        nc.sync.dma_start(plan_sb[:], plan[:])

        def plan_reg(eng, idx, max_val):
            tmp = eng.alloc_register(f"plan_{idx}_{nc.next_id()}")
            eng.reg_load(tmp, plan_sb[0:1, idx:idx + 1])
            return eng.snap(tmp, donate=True, min_val=0, max_val=max_val)

        # resident bf16 tensors: 10 part-tiles of (128, TPC) each
        QbT = [resid.tile([128, TPC], bf16, name=f"QbT{i}", tag=f"QbT{i}") for i in range(NCI)]
        QrT = [resid.tile([128, TPC], bf16, name=f"QrT{i}", tag=f"QrT{i}") for i in range(NCI)]
        ObT = [resid.tile([128, TPC], bf16, name=f"ObT{i}", tag=f"ObT{i}") for i in range(NCI)]
        OrT = [resid.tile([128, TPC], bf16, name=f"OrT{i}", tag=f"OrT{i}") for i in range(NCI)]
        OmT = [resid.tile([128, TPC], bf16, name=f"OmT{i}", tag=f"OmT{i}") for i in range(NCI)]

        KbL = dram.tile([C, TPC], bf16, tag="KbL"); KrL = dram.tile([C, TPC], bf16, tag="KrL")
        KmL = dram.tile([C, TPC], bf16, tag="KmL"); QmL = dram.tile([C, TPC], bf16, tag="QmL")
        VbL = dram.tile([TPC, C], bf16, tag="VbL"); VrL = dram.tile([TPC, C], bf16, tag="VrL")
        VmL = dram.tile([TPC, C], bf16, tag="VmL"); OmL = dram.tile([TPC, C], bf16, tag="OmL")
        KbG = dram.tile([4 * C, TPC], bf16, tag="KbG")
        KrG = dram.tile([4 * C, TPC], bf16, tag="KrG")
        VbG = dram.tile([HALF, C], bf16, tag="VbG")
        VrG = dram.tile([HALF, C], bf16, tag="VrG")
        KmG = dram.tile([NCORES * C, TPC], bf16, tag="KmG")
        QmG = dram.tile([NCORES * C, TPC], bf16, tag="QmG")
        VmG = dram.tile([TOK, C], bf16, tag="VmG")
        OmG = dram.tile([TOK, C], bf16, tag="OmG")

        G4 = [[0, 1, 2, 3], [4, 5, 6, 7]]
        G8 = [list(range(NCORES))]

        def copyback(dst_ap, src_ap, idx):
            if idx % 2:
                nc.vector.tensor_copy(dst_ap, src_ap)
            else:
                nc.scalar.copy(dst_ap, src_ap)

        # ============================ PHASE A ============================
        with tc.tile_pool(name="pXT", bufs=1) as pXT, \
             tc.tile_pool(name="pA", bufs=2) as pA, \
             tc.tile_pool(name="pAwf", bufs=2) as pAwf, \
             tc.tile_pool(name="pAwb", bufs=12) as pAwb, \
             tc.tile_pool(name="pAs", bufs=3) as pAs, \
             tc.tile_pool(name="psA", bufs=6, space="PSUM") as psA:

            XT = [pXT.tile([128, TPC], bf16, name=f"XT{i}", tag=f"XT{i}") for i in range(NCI)]
            RT = [pXT.tile([128, TPC], bf16, name=f"RT{i}", tag=f"RT{i}") for i in range(NCI)]
            with tc.tile_pool(name="psT", bufs=2, space="PSUM") as psT:
                for src, dstT in ((hid, XT), (ref, RT)):
                    for t in range(TPC // 128):
                        xn = pA.tile([128, C], bf16, tag="xnat")
                        nc.gpsimd.dma_start(xn[:], src[t * 128:(t + 1) * 128, :])
                        for ci in range(NCI):
                            tp = psT.tile([128, 128], bf16, tag="tp")
                            with nc.allow_low_precision(reason="bf16 transpose"):
                                nc.tensor.transpose(tp[:], xn[:, ci * 128:(ci + 1) * 128], ident[:])
                            copyback(dstT[ci][:, t * 128:(t + 1) * 128], tp[:], ci)

            def load_w_bf(poolf, poolb, wname, tag):
                tiles = []
                for ci in range(NCI):
                    wf = poolf.tile([128, C], f32, tag=f"wf_{tag}")
                    nc.scalar.dma_start(wf[:], Wn[wname][ci * 128:(ci + 1) * 128, :])
                    wb = poolb.tile([128, C], bf16, tag=f"wb_{tag}")
                    copyback(wb[:], wf[:], ci)
                    tiles.append(wb)
                return tiles

            def proj_T(wname, XTsrc, dest_sb=None, dest_dram=None):
                wt = load_w_bf(pAwf, pAwb, wname, "T")
                for co in range(NCI):
                    if dest_sb is not None:
                        stg = dest_sb[co]
                    else:
                        stg = pAs.tile([128, TPC], bf16, name="stgT", tag="stgT")
                    pss = [psA.tile([128, 512], f32, name=f"psT{k}", tag="psA")
                           for k in range(2)]
                    for ci in range(NCI):
                        for k in range(2):
                            nc.tensor.matmul(
                                pss[k][:, :384], wt[ci][:, co * 128:(co + 1) * 128],
                                XTsrc[ci][:, k * 384:(k + 1) * 384],
                                start=(ci == 0), stop=(ci == NCI - 1))
                    for k in range(2):
                        copyback(stg[:, k * 384:(k + 1) * 384], pss[k][:, :384], k)
                    if dest_dram is not None:
                        nc.sync.dma_start(dest_dram[co * 128:(co + 1) * 128, :], stg[:])

            def proj_N(wname, XTsrc, dest_dram):
                wt = load_w_bf(pAwf, pAwb, wname, "N")
                chunks = ((0, 512), (512, 1024), (1024, 1280))
                for t in range(TPC // 128):
                    stg = pAs.tile([128, C], bf16, tag="stgN")
                    pss = [psA.tile([128, 512], f32, name=f"psN{k}", tag="psA")
                           for k in range(3)]
                    for ci in range(NCI):
                        for k, (c0, c1) in enumerate(chunks):
                            nc.tensor.matmul(
                                pss[k][:, :c1 - c0], XTsrc[ci][:, t * 128:(t + 1) * 128],
                                wt[ci][:, c0:c1],
                                start=(ci == 0), stop=(ci == NCI - 1))
                    for k, (c0, c1) in enumerate(chunks):
                        copyback(stg[:, c0:c1], pss[k][:, :c1 - c0], k)
                    nc.sync.dma_start(dest_dram[t * 128:(t + 1) * 128, :], stg[:])

            def gather(t_in, t_out, groups):
                nc.gpsimd.collective_compute(
                    "AllGather", ALU.bypass, replica_groups=groups,
                    ins=[t_in[:].opt()], outs=[t_out[:].opt()])

            proj_T("Wk", XT, dest_dram=KbL); gather(KbL, KbG, G4)
            proj_N("Wv", XT, VbL); gather(VbL, VbG, G4)
            proj_T("Wq", XT, dest_sb=QbT)
            proj_T("Wk_ref", RT, dest_dram=KrL); gather(KrL, KrG, G4)
            proj_N("Wv_ref", RT, VrL); gather(VrL, VrG, G4)
            proj_T("Wq_ref", XT, dest_sb=QrT)
            proj_T("Wk_mv", XT, dest_dram=KmL); gather(KmL, KmG, G8)
            proj_T("Wq_mv", XT, dest_dram=QmL); gather(QmL, QmG, G8)
            proj_N("Wv_mv", XT, VmL); gather(VmL, VmG, G8)

        # ============================ PHASE B1: base + ref ============================
        with tc.tile_pool(name="pB", bufs=2) as pB, \
             tc.tile_pool(name="pBk", bufs=3) as pBk, \
             tc.tile_pool(name="psB", bufs=3, space="PSUM") as psB, \
             tc.tile_pool(name="psO", bufs=2, space="PSUM") as psO:

            KG = {"b": KbG, "r": KrG}
            VG = {"b": VbG, "r": VrG}
            QT_res = {"b": QbT, "r": QrT}
            OT_res = {"b": ObT, "r": OrT}

            for qc in range(NQC):
                krows = [plan_reg(nc.sync, qc * 4 + j, 3 * C) for j in range(4)]
                kcols = [plan_reg(nc.sync, 12 + qc * 4 + j, TPC - QC) for j in range(4)]
                vrow = plan_reg(nc.sync, 24 + qc, HALF - SEQ)
                for ty in ("b", "r"):
                    for hp in range(H // 2):
                        # K for head pair: even head in partitions 0-63,
                        # odd head in 64-127 (distinct PE row groups)
                        k_sb = pBk.tile([128, 4, QC], bf16, tag="k_sb")
                        for j in range(4):
                            nc.sync.dma_start(
                                k_sb[:, j, :],
                                KG[ty][bass.ds(krows[j] + hp * 2 * HD, 2 * HD),
                                       bass.ds(kcols[j], QC)])
                        # V pair layout [V0 | 1 | V1 | 1]: each head gets a
                        # contiguous [V | ones] 65-col lhsT, sum lands in row 64
                        v_sb = pB.tile([128, KT, 2 * HD + 2], bf16, tag="v_sb")
                        for e in range(2):
                            nc.sync.dma_start(
                                v_sb[:, :, e * (HD + 1):e * (HD + 1) + HD],
                                VG[ty][bass.ds(vrow, SEQ),
                                       (hp * 2 + e) * HD:(hp * 2 + e + 1) * HD]
                                .rearrange("(kt p) c -> p kt c", p=128))
                        nc.any.memset(v_sb[:, :, HD:HD + 1], 1.0)
                        nc.any.memset(v_sb[:, :, 2 * HD + 1:2 * HD + 2], 1.0)
                        a_sb = {}
                        for g in range(2):
                            s_ps = {}
                            for e in range(2):
                                s_ps[e] = psB.tile([128, 4, QC], f32,
                                                   name=f"s_ps{e}", tag="s_ps")
                            for kk in range(4):
                                kt = g * 4 + kk
                                for e in range(2):
                                    hb = e * 64
                                    nc.tensor.matmul(
                                        s_ps[e][:, kk, :],
                                        k_sb[hb:hb + 64, kt // 2,
                                             (kt % 2) * 128:(kt % 2) * 128 + 128],
                                        QT_res[ty][hp][hb:hb + 64,
                                                       qc * QC:(qc + 1) * QC],
                                        start=True, stop=True)
                            for e in range(2):
                                ab = pB.tile([128, 4, QC], bf16,
                                             name=f"a_sb{g}{e}", tag=f"a_sb{g}{e}")
                                nc.scalar.activation(
                                    ab[:].rearrange("p a b -> p (a b)"),
                                    s_ps[e][:].rearrange("p a b -> p (a b)"),
                                    AF.Exp, scale=SCALE)
                                a_sb[(g, e)] = ab
                        for e in range(2):
                            o_ps = psO.tile([HD + 1, QC], f32, tag="o_ps")
                            c0 = e * 65  # [0:65]=[1|V0], [65:130]=[V1|1]
                            for kt in range(KT):
                                nc.tensor.matmul(
                                    o_ps[:], v_sb[:, kt, c0:c0 + HD + 1],
                                    a_sb[(kt // 4, e)][:, kt % 4, :],
                                    start=(kt == 0), stop=(kt == KT - 1))
                            rec = pB.tile([1, QC], f32, tag="rec")
                            nc.vector.reciprocal(rec[:], o_ps[HD:HD + 1, :])
                            rep = pB.tile([HD, QC], f32, tag="rep")
                            nc.gpsimd.partition_broadcast(rep[:], rec[:])
                            nc.vector.tensor_tensor(
                                out=OT_res[ty][hp][e * 64:e * 64 + 64,
                                                   qc * QC:(qc + 1) * QC],
                                in0=o_ps[0:HD, :], in1=rep[:], op=ALU.mult)

        # ============================ PHASE B2: MV attention ============================
        with tc.tile_pool(name="pM", bufs=2) as pM, \
             tc.tile_pool(name="psM", bufs=2, space="PSUM") as psM:
            VmG_v = VmG[:].rearrange("(v q) c -> v q c", q=SEQ)
            for rl in range(4):
                mrows = [plan_reg(nc.sync, 27 + rl * NV + v, 7 * C) for v in range(NV)]
                mcols = [plan_reg(nc.sync, 51 + rl * NV + v, TPC - IW) for v in range(NV)]
                mvr = plan_reg(nc.sync, 75 + rl, SEQ - IW)
                mk = pM.tile([128, NCI, NV, IW], bf16, tag="mk")
                mq = pM.tile([128, NCI, NV, IW], bf16, tag="mq")
                for tl, GT in ((mk, KmG), (mq, QmG)):
                    for v in range(NV):
                        nc.sync.dma_start(
                            tl[:, :, v, :],
                            GT[bass.ds(mrows[v], C), bass.ds(mcols[v], IW)]
                            .rearrange("(ci p) b -> p ci b", p=128))
                mv0 = pM.tile([128, C], bf16, tag="mv0")
                nc.sync.dma_start(mv0[:], VmG_v[0:4, bass.ds(mvr, IW), :])
                mv1 = pM.tile([64, C], bf16, tag="mv1")
                nc.sync.dma_start(mv1[:], VmG_v[4:6, bass.ds(mvr, IW), :])
                for h in range(H):
                    kv = mk[(h % 2) * 64:(h % 2) * 64 + 64, h // 2, :, :] \
                        .rearrange("p v b -> p (v b)")
                    qv = mq[(h % 2) * 64:(h % 2) * 64 + 64, h // 2, :, :] \
                        .rearrange("p v b -> p (v b)")
                    s1 = psM.tile([128, LKV], f32, tag="ms1")
                    s2 = psM.tile([64, LKV], f32, tag="ms2")
                    nc.tensor.matmul(s1[:], kv[:, 0:128], qv[:], start=True, stop=True)
                    nc.tensor.matmul(s2[:], kv[:, 128:LKV], qv[:], start=True, stop=True)
                    a1 = pM.tile([128, LKV], bf16, tag="ma1")
                    a2 = pM.tile([64, LKV], bf16, tag="ma2")
                    nc.scalar.activation(a1[:], s1[:], AF.Exp, scale=SCALE)
                    nc.scalar.activation(a2[:], s2[:], AF.Exp, scale=SCALE)
                    o_ps = psM.tile([HD, LKV], f32, tag="mo")
                    nc.tensor.matmul(o_ps[:], mv0[:, h * HD:(h + 1) * HD], a1[:],
                                     start=True, stop=False)
                    nc.tensor.matmul(o_ps[:], mv1[:, h * HD:(h + 1) * HD], a2[:],
                                     start=False, stop=True)
                    r_ps = psM.tile([1, LKV], f32, tag="mr")
                    nc.tensor.matmul(r_ps[:], ones_col[:, 0:1], a1[:],
                                     start=True, stop=False)
                    nc.tensor.matmul(r_ps[:], ones_col[0:64, 0:1], a2[:],
                                     start=False, stop=True)
                    rec = pM.tile([1, LKV], f32, tag="mrec")
                    nc.vector.reciprocal(rec[:], r_ps[:])
                    rep = pM.tile([HD, LKV], f32, tag="mrep")
                    nc.gpsimd.partition_broadcast(rep[:], rec[:])
                    nc.vector.tensor_tensor(
                        out=OmT[h // 2][(h % 2) * 64:(h % 2) * 64 + 64,
                                        rl * LKV:(rl + 1) * LKV],
                        in0=o_ps[:], in1=rep[:], op=ALU.mult)

        # MV out-projection over local rows, then gather
        with tc.tile_pool(name="pMP", bufs=3) as pMP, \
             tc.tile_pool(name="pMPf", bufs=2) as pMPf, \
             tc.tile_pool(name="pMPb", bufs=10) as pMPb, \
             tc.tile_pool(name="psMP", bufs=4, space="PSUM") as psMP:
            wt = []
            for ci in range(NCI):
                wf = pMPf.tile([128, C], f32, tag="mw_f")
                nc.scalar.dma_start(wf[:], Wn["Wout_mv"][ci * 128:(ci + 1) * 128, :])
                wb = pMPb.tile([128, C], bf16, tag="mw_b")
                copyback(wb[:], wf[:], ci)
                wt.append(wb)
            for t in range(TPC // 128):
                stg = pMP.tile([128, C], bf16, tag="m_stg")
                for k, (c0, c1) in enumerate(((0, 512), (512, 1024), (1024, 1280))):
                    ps = psMP.tile([128, 512], f32, tag="psMP")
                    for ci in range(NCI):
                        nc.tensor.matmul(ps[:, :c1 - c0],
                                         OmT[ci][:, t * 128:(t + 1) * 128],
                                         wt[ci][:, c0:c1],
                                         start=(ci == 0), stop=(ci == NCI - 1))
                    copyback(stg[:, c0:c1], ps[:, :c1 - c0], k)
                nc.sync.dma_start(OmL[t * 128:(t + 1) * 128, :], stg[:])
            nc.gpsimd.collective_compute(
                "AllGather", ALU.bypass, replica_groups=G8,
                ins=[OmL[:].opt()], outs=[OmG[:].opt()])

        # ============================ PHASE C ============================
        with tc.tile_pool(name="pC", bufs=2) as pC, \
             tc.tile_pool(name="pCwf", bufs=2) as pCwf, \
             tc.tile_pool(name="pCwb", bufs=10) as pCwb, \
             tc.tile_pool(name="psC", bufs=4, space="PSUM") as psC:
            wts = {}
            for nm in ("Wout", "Wout_ref"):
                tl = []
                for ci in range(NCI):
                    wf = pCwf.tile([128, C], f32, tag="cw_f")
                    nc.scalar.dma_start(wf[:], Wn[nm][ci * 128:(ci + 1) * 128, :])
                    wb = pCwb.tile([128, C], bf16, tag=f"cw_b_{nm}")
                    copyback(wb[:], wf[:], ci)
                    tl.append(wb)
                wts[nm] = tl
            for t in range(TPC // 128):
                res_t = pC.tile([128, C], f32, tag="res")
                nc.sync.dma_start(res_t[:], hid[t * 128:(t + 1) * 128, :])
                mv_t = pC.tile([128, C], f32, tag="mvt")
                for j in range(4):
                    mo = plan_reg(nc.gpsimd, 79 + t * 4 + j, TOK - IW)
                    nc.gpsimd.dma_start(mv_t[j * IW:(j + 1) * IW, :],
                                        OmG[bass.ds(mo, IW), :])
                out_t = pC.tile([128, C], f32, tag="outt")
                chunksC = ((0, 512), (512, 1024), (1024, 1280))
                pss = [psC.tile([128, 512], f32, name=f"psC{k}", tag="psC")
                       for k in range(3)]
                first = True
                for srcT, wnm in ((ObT, "Wout"), (OrT, "Wout_ref")):
                    for ci in range(NCI):
                        for k, (c0, c1) in enumerate(chunksC):
                            nc.tensor.matmul(pss[k][:, :c1 - c0],
                                             srcT[ci][:, t * 128:(t + 1) * 128],
                                             wts[wnm][ci][:, c0:c1],
                                             start=first, stop=False)
                        first = False
                for k, (c0, c1) in enumerate(chunksC):
                    nc.tensor.matmul(pss[k][:, :c1 - c0], ones_row[:],
                                     bsum_bf[0:1, c0:c1], start=False, stop=True)
                    t1 = pC.tile([128, 512], f32, tag="t1")
                    nc.vector.tensor_tensor(out=t1[:, :c1 - c0], in0=pss[k][:, :c1 - c0],
                                            in1=res_t[:, c0:c1], op=ALU.add)
                    nc.vector.tensor_tensor(out=out_t[:, c0:c1], in0=t1[:, :c1 - c0],
                                            in1=mv_t[:, c0:c1], op=ALU.add)
                nc.sync.dma_start(out[t * 128:(t + 1) * 128, :], out_t[:])

    nc.compile()
    return nc


def _plans():
    plans = []
    for c in range(NCORES):
        p = np.zeros(PLAN_LEN, dtype=np.uint32)
        g = c // 4
        for qc in range(NQC):
            t0 = c * TPC + qc * QC
            bt = t0 // SEQ - 3 * g
            for j in range(4):
                hcol = bt * SEQ + QC * j
                rank, col = hcol // TPC, hcol % TPC
                p[qc * 4 + j] = rank * C
                p[12 + qc * 4 + j] = col
            p[24 + qc] = bt * SEQ
        for rl in range(4):
            r_gl = 4 * c + rl
            for v in range(NV):
                tk = v * SEQ + IW * r_gl
                rank, col = tk // TPC, tk % TPC
                p[27 + rl * NV + v] = rank * C
                p[51 + rl * NV + v] = col
            p[75 + rl] = IW * r_gl
        for tt in range(TPC // 128):
            for j in range(4):
                t0 = c * TPC + tt * 128 + j * IW
                v, rem = divmod(t0, SEQ)
                rblk = rem // IW
                p[79 + tt * 4 + j] = (rblk // 4) * TPC + (rblk % 4) * LKV + v * IW
        plans.append(p.reshape(1, PLAN_LEN))
    return plans


def kernel(**inputs):
    if "nc" not in _CACHE:
        _CACHE["nc"] = _build()
        _CACHE["plans"] = _plans()
    nc = _CACHE["nc"]
    hid = np.asarray(inputs["hidden_states"], dtype=np.float32).reshape(TOK, C)
    ref = np.asarray(inputs["ref_hidden_states"], dtype=np.float32).reshape(TOK, C)
    bsum = (np.asarray(inputs["bout"]) + np.asarray(inputs["bout_mv"])
            + np.asarray(inputs["bout_ref"])).astype(np.float32).reshape(1, C)
    in_maps = []
    for c in range(NCORES):
        m = {
            "hid_shard": np.ascontiguousarray(hid[c * TPC:(c + 1) * TPC]),
            "ref_shard": np.ascontiguousarray(ref[c * TPC:(c + 1) * TPC]),
            "bsum": bsum,
            "plan": _CACHE["plans"][c],
        }
        for n in ["Wq", "Wk", "Wv", "Wq_mv", "Wk_mv", "Wv_mv",
                  "Wq_ref", "Wk_ref", "Wv_ref", "Wout", "Wout_mv", "Wout_ref"]:
            m[n] = np.ascontiguousarray(np.asarray(inputs[n], dtype=np.float32))
        in_maps.append(m)
    res = run_bass_kernel_spmd(nc, in_maps, list(range(NCORES)))
    full = np.concatenate([res.results[c]["out_shard"] for c in range(NCORES)], axis=0)
    return full.reshape(BS, SEQ, C)


if __name__ == "__main__":
    _build()
    print("BUILD OK")



# revision 2
# speedup vs baseline: 1.0006x; 1.0006x over previous
"""Trainium2 Bass kernel for DecoupledMVRowSelfAttnProcessor (bs=6, seq=1024, C=1280,
20 heads, 6 views, row-wise MV attention). Self-contained: builds + compiles an 8-core
SPMD Tile kernel on first call, then runs it via run_bass_kernel_spmd.

v2 layout:
  Weights arrive pre-converted to bf16 (host-side), halving weight HBM traffic and
  removing on-device fp32->bf16 conversion work.
  All gathers are 8-core AllGathers with addr_space="Shared" outputs: the gathered
  tensor is written once to shared HBM instead of once per core (the compiler's
  recommended fast path), and each core reads back only the slices it needs.
  V tensors are staged in a padded head-major [V_h | 1] 65-column layout so the
  softmax denominator rides along row 64 of the A@V matmul, and phase-B V loads are
  one large contiguous DMA per (qc, ty) (>=512B descriptors, no RMW penalty).

Phases:
  A (token-sharded, 768 tok/core): X^T via PE-transpose, 9 QKV projections in bf16;
    K^T / V written to local DRAM and AllGathered (shared outputs); Q^T for base/ref
    stays SBUF-resident.
  B1 (query-sharded base/ref): softmax over the kv partition axis (exp -> embedded
    ones-column row sums -> reciprocal -> partition_broadcast); O^T accumulates
    SBUF-resident.
  B2 (image-row-sharded MV attention) + MV out-projection, rows AllGathered.
  C (token-sharded): base/ref out-projections + combined bias + MV rows + residual.
"""
import sys
sys.path.insert(0, '/opt/trn_rl_repo')
import contextlib
import math
import numpy as np

import concourse.bass as bass
import concourse.mybir as mybir
from concourse import bacc
from concourse.tile import TileContext
from concourse.bass_utils import run_bass_kernel_spmd
from concourse.masks import make_identity

f32 = mybir.dt.float32
bf16 = mybir.dt.bfloat16
u32 = mybir.dt.uint32
AF = mybir.ActivationFunctionType
ALU = mybir.AluOpType

NCORES = 8
BS, SEQ, C = 6, 1024, 1280
H, HD, NV = 20, 64, 6
IH = IW = 32
TOK = BS * SEQ            # 6144
TPC = TOK // NCORES       # 768
NCI = C // 128            # 10
QC = 256                  # query chunk (always batch-pure)
NQC = TPC // QC           # 3
KT = SEQ // 128           # 8
LKV = NV * IW             # 192
HDP = HD + 1              # 65: [V_h | 1] per-head stride in padded V layout
CV = H * HDP              # 1300: padded V row width
SCALE = 1.0 / math.sqrt(HD)

WNAMES = ["Wq", "Wk", "Wv", "Wq_mv", "Wk_mv", "Wv_mv", "Wq_ref", "Wk_ref",
          "Wv_ref", "Wout", "Wout_mv", "Wout_ref"]

# plan tensor layout (uint32, per core):
#   [0:12]   krow[qc*4+j]  row base (rank*C) into K{b,r}G (8C, TPC)
#   [12:24]  kcol[qc*4+j]  col base into K{b,r}G
#   [24:27]  vrow[qc]      row base (b*SEQ) into V{b,r}G (TOK, CV)
#   [27:51]  mrow[rl*6+v]  row base (rank*C) into {K,Q}mG (8C, TPC)
#   [51:75]  mcol[rl*6+v]  col base into {K,Q}mG
#   [75:79]  mvr[rl]       row base (r_gl*IW) within each SEQ block of VmG
#   [79:103] crow[tt*4+j]  row base into OmG (TOK, C)
PLAN_LEN = 128

_CACHE = {}


def _build():
    nc = bacc.Bacc("TRN2", target_bir_lowering=False, debug=False, num_devices=NCORES)

    hid = nc.declare_dram_parameter("hid_shard", [TPC, C], f32, isOutput=False)
    ref = nc.declare_dram_parameter("ref_shard", [TPC, C], f32, isOutput=False)
    Wn = {n: nc.declare_dram_parameter(n, [C, C], bf16, isOutput=False) for n in WNAMES}
    bsum = nc.declare_dram_parameter("bsum", [1, C], f32, isOutput=False)
    plan = nc.declare_dram_parameter("plan", [1, PLAN_LEN], u32, isOutput=False)
    out = nc.declare_dram_parameter("out_shard", [TPC, C], f32, isOutput=True)

    with TileContext(nc) as tc, contextlib.ExitStack() as stack:
        const = stack.enter_context(tc.tile_pool(name="const", bufs=1))
        resid = stack.enter_context(tc.tile_pool(name="resident", bufs=1))
        dram = stack.enter_context(tc.tile_pool(name="dram", bufs=1, space="DRAM"))

        ident = const.tile([128, 128], bf16)
        make_identity(nc, ident[:])
        ones_row = const.tile([1, 128], bf16)   # lhsT for bias broadcast
        nc.any.memset(ones_row[:], 1.0)
        bsum_bf = const.tile([1, C], bf16)
        nc.gpsimd.dma_start(bsum_bf[:], bsum[:])
        plan_sb = const.tile([1, PLAN_LEN], u32)
        nc.sync.dma_start(plan_sb[:], plan[:])

        def plan_reg(eng, idx, max_val):
            tmp = eng.alloc_register(f"plan_{idx}_{nc.next_id()}")
            eng.reg_load(tmp, plan_sb[0:1, idx:idx + 1])
            return eng.snap(tmp, donate=True, min_val=0, max_val=max_val)

        # resident bf16 tensors: 10 part-tiles of (128, TPC) each
        QbT = [resid.tile([128, TPC], bf16, name=f"QbT{i}", tag=f"QbT{i}") for i in range(NCI)]
        QrT = [resid.tile([128, TPC], bf16, name=f"QrT{i}", tag=f"QrT{i}") for i in range(NCI)]
        ObT = [resid.tile([128, TPC], bf16, name=f"ObT{i}", tag=f"ObT{i}") for i in range(NCI)]
        OrT = [resid.tile([128, TPC], bf16, name=f"OrT{i}", tag=f"OrT{i}") for i in range(NCI)]
        OmT = [resid.tile([128, TPC], bf16, name=f"OmT{i}", tag=f"OmT{i}") for i in range(NCI)]

        KbL = dram.tile([C, TPC], bf16, tag="KbL"); KrL = dram.tile([C, TPC], bf16, tag="KrL")
        KmL = dram.tile([C, TPC], bf16, tag="KmL"); QmL = dram.tile([C, TPC], bf16, tag="QmL")
        VbL = dram.tile([TPC, CV], bf16, tag="VbL"); VrL = dram.tile([TPC, CV], bf16, tag="VrL")
        VmL = dram.tile([TPC, CV], bf16, tag="VmL"); OmL = dram.tile([TPC, C], bf16, tag="OmL")
        KbG = dram.tile([NCORES * C, TPC], bf16, tag="KbG", addr_space="Shared")
        KrG = dram.tile([NCORES * C, TPC], bf16, tag="KrG", addr_space="Shared")
        VbG = dram.tile([TOK, CV], bf16, tag="VbG", addr_space="Shared")
        VrG = dram.tile([TOK, CV], bf16, tag="VrG", addr_space="Shared")
        KmG = dram.tile([NCORES * C, TPC], bf16, tag="KmG", addr_space="Shared")
        QmG = dram.tile([NCORES * C, TPC], bf16, tag="QmG", addr_space="Shared")
        VmG = dram.tile([TOK, CV], bf16, tag="VmG", addr_space="Shared")
        OmG = dram.tile([TOK, C], bf16, tag="OmG", addr_space="Shared")

        G8 = [list(range(NCORES))]

        def copyback(dst_ap, src_ap, idx):
            if idx % 2:
                nc.vector.tensor_copy(dst_ap, src_ap)
            else:
                nc.scalar.copy(dst_ap, src_ap)

        # ============================ PHASE A ============================
        with tc.tile_pool(name="pXT", bufs=1) as pXT, \
             tc.tile_pool(name="pA", bufs=2) as pA, \
             tc.tile_pool(name="pAwb", bufs=12) as pAwb, \
             tc.tile_pool(name="pAs", bufs=3) as pAs, \
             tc.tile_pool(name="psA", bufs=6, space="PSUM") as psA:

            XT = [pXT.tile([128, TPC], bf16, name=f"XT{i}", tag=f"XT{i}") for i in range(NCI)]
            RT = [pXT.tile([128, TPC], bf16, name=f"RT{i}", tag=f"RT{i}") for i in range(NCI)]
            with tc.tile_pool(name="psT", bufs=2, space="PSUM") as psT:
                for src, dstT in ((hid, XT), (ref, RT)):
                    for t in range(TPC // 128):
                        xn = pA.tile([128, C], bf16, tag="xnat")
                        nc.gpsimd.dma_start(xn[:], src[t * 128:(t + 1) * 128, :])
                        for ci in range(NCI):
                            tp = psT.tile([128, 128], bf16, tag="tp")
                            with nc.allow_low_precision(reason="bf16 transpose"):
                                nc.tensor.transpose(tp[:], xn[:, ci * 128:(ci + 1) * 128], ident[:])
                            copyback(dstT[ci][:, t * 128:(t + 1) * 128], tp[:], ci)

            def load_w_bf(wname, tag):
                tiles = []
                for ci in range(NCI):
                    wb = pAwb.tile([128, C], bf16, tag=f"wb_{tag}")
                    nc.scalar.dma_start(wb[:], Wn[wname][ci * 128:(ci + 1) * 128, :])
                    tiles.append(wb)
                return tiles

            def proj_T(wname, XTsrc, dest_sb=None, dest_dram=None):
                wt = load_w_bf(wname, "T")
                for co in range(NCI):
                    if dest_sb is not None:
                        stg = dest_sb[co]
                    else:
                        stg = pAs.tile([128, TPC], bf16, name="stgT", tag="stgT")
                    pss = [psA.tile([128, 512], f32, name=f"psT{k}", tag="psA")
                           for k in range(2)]
                    for ci in range(NCI):
                        for k in range(2):
                            nc.tensor.matmul(
                                pss[k][:, :384], wt[ci][:, co * 128:(co + 1) * 128],
                                XTsrc[ci][:, k * 384:(k + 1) * 384],
                                start=(ci == 0), stop=(ci == NCI - 1))
                    for k in range(2):
                        copyback(stg[:, k * 384:(k + 1) * 384], pss[k][:, :384], k)
                    if dest_dram is not None:
                        nc.sync.dma_start(dest_dram[co * 128:(co + 1) * 128, :], stg[:])

            def proj_V(wname, XTsrc, dest_dram):
                # out rows in padded [V_h | 1] layout: head h at cols h*65..h*65+64
                wt = load_w_bf(wname, "N")
                chunks = ((0, 512, 0, 8), (512, 1024, 8, 16), (1024, 1280, 16, 20))
                for t in range(TPC // 128):
                    stg = pAs.tile([128, CV], bf16, tag="stgV")
                    stg_h = stg[:].rearrange("p (h c) -> p h c", c=HDP)
                    nc.any.memset(stg_h[:, :, HD:HDP], 1.0)
                    pss = [psA.tile([128, 512], f32, name=f"psN{k}", tag="psA")
                           for k in range(3)]
                    for ci in range(NCI):
                        for k, (c0, c1, _, _) in enumerate(chunks):
                            nc.tensor.matmul(
                                pss[k][:, :c1 - c0], XTsrc[ci][:, t * 128:(t + 1) * 128],
                                wt[ci][:, c0:c1],
                                start=(ci == 0), stop=(ci == NCI - 1))
                    for k, (c0, c1, h0, h1) in enumerate(chunks):
                        copyback(
                            stg_h[:, h0:h1, 0:HD],
                            pss[k][:, :c1 - c0].rearrange("p (h c) -> p h c", c=HD), k)
                    nc.sync.dma_start(dest_dram[t * 128:(t + 1) * 128, :], stg[:])

            def gather(t_in, t_out):
                nc.gpsimd.collective_compute(
                    "AllGather", ALU.bypass, replica_groups=G8,
                    ins=[t_in[:].opt()], outs=[t_out[:].opt()])

            proj_T("Wk", XT, dest_dram=KbL); gather(KbL, KbG)
            proj_V("Wv", XT, VbL); gather(VbL, VbG)
            proj_T("Wk_ref", RT, dest_dram=KrL); gather(KrL, KrG)
            proj_V("Wv_ref", RT, VrL); gather(VrL, VrG)
            proj_T("Wq", XT, dest_sb=QbT)
            proj_T("Wq_ref", XT, dest_sb=QrT)
            proj_T("Wk_mv", XT, dest_dram=KmL); gather(KmL, KmG)
            proj_T("Wq_mv", XT, dest_dram=QmL); gather(QmL, QmG)
            proj_V("Wv_mv", XT, VmL); gather(VmL, VmG)

        # ============================ PHASE B1: base + ref ============================
        with tc.tile_pool(name="pB", bufs=2) as pB, \
             tc.tile_pool(name="pBk", bufs=2) as pBk, \
             tc.tile_pool(name="pBv", bufs=2) as pBv, \
             tc.tile_pool(name="psB", bufs=3, space="PSUM") as psB, \
             tc.tile_pool(name="psO", bufs=2, space="PSUM") as psO:

            KG = {"b": KbG, "r": KrG}
            VG = {"b": VbG, "r": VrG}
            QT_res = {"b": QbT, "r": QrT}
            OT_res = {"b": ObT, "r": OrT}

            for qc in range(NQC):
                krows = [plan_reg(nc.sync, qc * 4 + j, 7 * C) for j in range(4)]
                kcols = [plan_reg(nc.sync, 12 + qc * 4 + j, TPC - QC) for j in range(4)]
                vrow = plan_reg(nc.sync, 24 + qc, TOK - SEQ)
                for ty in ("b", "r"):
                    # K^T for all channels: [128, ci, j, QC]; head h lives at
                    # partitions (h%2)*64.. of slab ci=h//2
                    k_sb = pBk.tile([128, NCI, 4, QC], bf16, tag="k_sb")
                    for j in range(4):
                        nc.sync.dma_start(
                            k_sb[:, :, j, :],
                            KG[ty][bass.ds(krows[j], C), bass.ds(kcols[j], QC)]
                            .rearrange("(ci p) c -> p ci c", p=128))
                    # V rows for this batch in padded [V_h | 1] layout
                    v_sb = pBv.tile([128, KT, CV], bf16, tag="v_sb")
                    nc.sync.dma_start(
                        v_sb[:],
                        VG[ty][bass.ds(vrow, SEQ), :]
                        .rearrange("(kt p) c -> p kt c", p=128))
                    for hp in range(H // 2):
                        a_sb = {}
                        for g in range(2):
                            s_ps = {}
                            for e in range(2):
                                s_ps[e] = psB.tile([128, 4, QC], f32,
                                                   name=f"s_ps{e}", tag="s_ps")
                            for kk in range(4):
                                kt = g * 4 + kk
                                for e in range(2):
                                    hb = e * 64
                                    nc.tensor.matmul(
                                        s_ps[e][:, kk, :],
                                        k_sb[hb:hb + 64, hp, kt // 2,
                                             (kt % 2) * 128:(kt % 2) * 128 + 128],
                                        QT_res[ty][hp][hb:hb + 64,
                                                       qc * QC:(qc + 1) * QC],
                                        start=True, stop=True)
                            for e in range(2):
                                ab = pB.tile([128, 4, QC], bf16,
                                             name=f"a_sb{g}{e}", tag=f"a_sb{g}{e}")
                                nc.scalar.activation(
                                    ab[:].rearrange("p a b -> p (a b)"),
                                    s_ps[e][:].rearrange("p a b -> p (a b)"),
                                    AF.Exp, scale=SCALE)
                                a_sb[(g, e)] = ab
                        for e in range(2):
                            h = hp * 2 + e
                            o_ps = psO.tile([HDP, QC], f32, tag="o_ps")
                            for kt in range(KT):
                                nc.tensor.matmul(
                                    o_ps[:], v_sb[:, kt, h * HDP:(h + 1) * HDP],
                                    a_sb[(kt // 4, e)][:, kt % 4, :],
                                    start=(kt == 0), stop=(kt == KT - 1))
                            rec = pB.tile([1, QC], f32, tag="rec")
                            nc.vector.reciprocal(rec[:], o_ps[HD:HDP, :])
                            rep = pB.tile([HD, QC], f32, tag="rep")
                            nc.gpsimd.partition_broadcast(rep[:], rec[:])
                            nc.vector.tensor_tensor(
                                out=OT_res[ty][hp][e * 64:e * 64 + 64,
                                                   qc * QC:(qc + 1) * QC],
                                in0=o_ps[0:HD, :], in1=rep[:], op=ALU.mult)

        # ============================ PHASE B2: MV attention ============================
        with tc.tile_pool(name="pM", bufs=2) as pM, \
             tc.tile_pool(name="psM", bufs=2, space="PSUM") as psM:
            VmG_v = VmG[:].rearrange("(v q) c -> v q c", q=SEQ)
            for rl in range(4):
                mrows = [plan_reg(nc.sync, 27 + rl * NV + v, 7 * C) for v in range(NV)]
                mcols = [plan_reg(nc.sync, 51 + rl * NV + v, TPC - IW) for v in range(NV)]
                mvr = plan_reg(nc.sync, 75 + rl, SEQ - IW)
                mk = pM.tile([128, NCI, NV, IW], bf16, tag="mk")
                mq = pM.tile([128, NCI, NV, IW], bf16, tag="mq")
                for tl, GT in ((mk, KmG), (mq, QmG)):
                    for v in range(NV):
                        nc.sync.dma_start(
                            tl[:, :, v, :],
                            GT[bass.ds(mrows[v], C), bass.ds(mcols[v], IW)]
                            .rearrange("(ci p) b -> p ci b", p=128))
                mv0 = pM.tile([128, CV], bf16, tag="mv0")
                nc.sync.dma_start(mv0[:], VmG_v[0:4, bass.ds(mvr, IW), :])
                mv1 = pM.tile([64, CV], bf16, tag="mv1")
                nc.sync.dma_start(mv1[:], VmG_v[4:6, bass.ds(mvr, IW), :])
                for h in range(H):
                    kv = mk[(h % 2) * 64:(h % 2) * 64 + 64, h // 2, :, :] \
                        .rearrange("p v b -> p (v b)")
                    qv = mq[(h % 2) * 64:(h % 2) * 64 + 64, h // 2, :, :] \
                        .rearrange("p v b -> p (v b)")
                    s1 = psM.tile([128, LKV], f32, tag="ms1")
                    s2 = psM.tile([64, LKV], f32, tag="ms2")
                    nc.tensor.matmul(s1[:], kv[:, 0:128], qv[:], start=True, stop=True)
                    nc.tensor.matmul(s2[:], kv[:, 128:LKV], qv[:], start=True, stop=True)
                    a1 = pM.tile([128, LKV], bf16, tag="ma1")
                    a2 = pM.tile([64, LKV], bf16, tag="ma2")
                    nc.scalar.activation(a1[:], s1[:], AF.Exp, scale=SCALE)
                    nc.scalar.activation(a2[:], s2[:], AF.Exp, scale=SCALE)
                    o_ps = psM.tile([HDP, LKV], f32, tag="mo")
                    nc.tensor.matmul(o_ps[:], mv0[:, h * HDP:(h + 1) * HDP], a1[:],
                                     start=True, stop=False)
                    nc.tensor.matmul(o_ps[:], mv1[:, h * HDP:(h + 1) * HDP], a2[:],
                                     start=False, stop=True)
                    rec = pM.tile([1, LKV], f32, tag="mrec")
                    nc.vector.reciprocal(rec[:], o_ps[HD:HDP, :])
                    rep = pM.tile([HD, LKV], f32, tag="mrep")
                    nc.gpsimd.partition_broadcast(rep[:], rec[:])
                    nc.vector.tensor_tensor(
                        out=OmT[h // 2][(h % 2) * 64:(h % 2) * 64 + 64,
                                        rl * LKV:(rl + 1) * LKV],
                        in0=o_ps[0:HD, :], in1=rep[:], op=ALU.mult)

        # MV out-projection over local rows, then gather
        with tc.tile_pool(name="pMP", bufs=3) as pMP, \
             tc.tile_pool(name="pMPb", bufs=10) as pMPb, \
             tc.tile_pool(name="psMP", bufs=4, space="PSUM") as psMP:
            wt = []
            for ci in range(NCI):
                wb = pMPb.tile([128, C], bf16, tag="mw_b")
                nc.scalar.dma_start(wb[:], Wn["Wout_mv"][ci * 128:(ci + 1) * 128, :])
                wt.append(wb)
            for t in range(TPC // 128):
                stg = pMP.tile([128, C], bf16, tag="m_stg")
                for k, (c0, c1) in enumerate(((0, 512), (512, 1024), (1024, 1280))):
                    ps = psMP.tile([128, 512], f32, tag="psMP")
                    for ci in range(NCI):
                        nc.tensor.matmul(ps[:, :c1 - c0],
                                         OmT[ci][:, t * 128:(t + 1) * 128],
                                         wt[ci][:, c0:c1],
                                         start=(ci == 0), stop=(ci == NCI - 1))
                    copyback(stg[:, c0:c1], ps[:, :c1 - c0], k)
                nc.sync.dma_start(OmL[t * 128:(t + 1) * 128, :], stg[:])
            nc.gpsimd.collective_compute(
                "AllGather", ALU.bypass, replica_groups=G8,
                ins=[OmL[:].opt()], outs=[OmG[:].opt()])

        # ============================ PHASE C ============================
        with tc.tile_pool(name="pC", bufs=2) as pC, \
             tc.tile_pool(name="pCwb", bufs=10) as pCwb, \
             tc.tile_pool(name="psC", bufs=4, space="PSUM") as psC:
            wts = {}
            for nm in ("Wout", "Wout_ref"):
                tl = []
                for ci in range(NCI):
                    wb = pCwb.tile([128, C], bf16, tag=f"cw_b_{nm}")
                    nc.scalar.dma_start(wb[:], Wn[nm][ci * 128:(ci + 1) * 128, :])
                    tl.append(wb)
                wts[nm] = tl
            for t in range(TPC // 128):
                res_t = pC.tile([128, C], f32, tag="res")
                nc.sync.dma_start(res_t[:], hid[t * 128:(t + 1) * 128, :])
                mv_t = pC.tile([128, C], f32, tag="mvt")
                for j in range(4):
                    mo = plan_reg(nc.gpsimd, 79 + t * 4 + j, TOK - IW)
                    nc.gpsimd.dma_start(mv_t[j * IW:(j + 1) * IW, :],
                                        OmG[bass.ds(mo, IW), :])
                out_t = pC.tile([128, C], f32, tag="outt")
                chunksC = ((0, 512), (512, 1024), (1024, 1280))
                pss = [psC.tile([128, 512], f32, name=f"psC{k}", tag="psC")
                       for k in range(3)]
                first = True
                for srcT, wnm in ((ObT, "Wout"), (OrT, "Wout_ref")):
                    for ci in range(NCI):
                        for k, (c0, c1) in enumerate(chunksC):
                            nc.tensor.matmul(pss[k][:, :c1 - c0],
                                             srcT[ci][:, t * 128:(t + 1) * 128],
                                             wts[wnm][ci][:, c0:c1],
                                             start=first, stop=False)
                        first = False
                for k, (c0, c1) in enumerate(chunksC):
                    nc.tensor.matmul(pss[k][:, :c1 - c0], ones_row[:],
                                     bsum_bf[0:1, c0:c1], start=False, stop=True)
                    t1 = pC.tile([128, 512], f32, tag="t1")
                    nc.vector.tensor_tensor(out=t1[:, :c1 - c0], in0=pss[k][:, :c1 - c0],
                                            in1=res_t[:, c0:c1], op=ALU.add)
                    nc.vector.tensor_tensor(out=out_t[:, c0:c1], in0=t1[:, :c1 - c0],
                                            in1=mv_t[:, c0:c1], op=ALU.add)
                nc.sync.dma_start(out[t * 128:(t + 1) * 128, :], out_t[:])

    nc.compile()
    return nc


def _plans():
    plans = []
    for c in range(NCORES):
        p = np.zeros(PLAN_LEN, dtype=np.uint32)
        for qc in range(NQC):
            t0 = c * TPC + qc * QC
            b = t0 // SEQ
            for j in range(4):
                hcol = b * SEQ + QC * j
                rank, col = hcol // TPC, hcol % TPC
                p[qc * 4 + j] = rank * C
                p[12 + qc * 4 + j] = col
            p[24 + qc] = b * SEQ
        for rl in range(4):
            r_gl = 4 * c + rl
            for v in range(NV):
                tk = v * SEQ + IW * r_gl
                rank, col = tk // TPC, tk % TPC
                p[27 + rl * NV + v] = rank * C
                p[51 + rl * NV + v] = col
            p[75 + rl] = IW * r_gl
        for tt in range(TPC // 128):
            for j in range(4):
                t0 = c * TPC + tt * 128 + j * IW
                v, rem = divmod(t0, SEQ)
                rblk = rem // IW
                p[79 + tt * 4 + j] = (rblk // 4) * TPC + (rblk % 4) * LKV + v * IW
        plans.append(p.reshape(1, PLAN_LEN))
    return plans


def _in_maps(inputs):
    import ml_dtypes
    hid = np.asarray(inputs["hidden_states"], dtype=np.float32).reshape(TOK, C)
    ref = np.asarray(inputs["ref_hidden_states"], dtype=np.float32).reshape(TOK, C)
    bsum = (np.asarray(inputs["bout"]) + np.asarray(inputs["bout_mv"])
            + np.asarray(inputs["bout_ref"])).astype(np.float32).reshape(1, C)
    wbf = {n: np.ascontiguousarray(
        np.asarray(inputs[n], dtype=np.float32).astype(ml_dtypes.bfloat16))
        for n in WNAMES}
    plans = _plans()
    in_maps = []
    for c in range(NCORES):
        m = {
            "hid_shard": np.ascontiguousarray(hid[c * TPC:(c + 1) * TPC]),
            "ref_shard": np.ascontiguousarray(ref[c * TPC:(c + 1) * TPC]),
            "bsum": bsum,
            "plan": plans[c],
        }
        m.update(wbf)
        in_maps.append(m)
    return in_maps


def kernel(**inputs):
    if "nc" not in _CACHE:
        _CACHE["nc"] = _build()
    nc = _CACHE["nc"]
    res = run_bass_kernel_spmd(nc, _in_maps(inputs), list(range(NCORES)))
    full = np.concatenate([res.results[c]["out_shard"] for c in range(NCORES)], axis=0)
    return full.reshape(BS, SEQ, C)


if __name__ == "__main__":
    _build()
    print("BUILD OK")


# revision 10
# speedup vs baseline: 1.4982x; 1.4972x over previous
"""Trainium2 Bass kernel for DecoupledMVRowSelfAttnProcessor (bs=6, seq=1024, C=1280,
20 heads, 6 views, row-wise MV attention). Self-contained: builds + compiles an 8-core
SPMD Tile kernel on first call, then runs it via run_bass_kernel_spmd.

v2 layout:
  Weights arrive pre-converted to bf16 (host-side), halving weight HBM traffic and
  removing on-device fp32->bf16 conversion work.
  All gathers are 8-core AllGathers with addr_space="Shared" outputs: the gathered
  tensor is written once to shared HBM instead of once per core (the compiler's
  recommended fast path), and each core reads back only the slices it needs.
  V tensors are staged in a padded head-major [V_h | 1] 65-column layout so the
  softmax denominator rides along row 64 of the A@V matmul, and phase-B V loads are
  one large contiguous DMA per (qc, ty) (>=512B descriptors, no RMW penalty).

Phases:
  A (token-sharded, 768 tok/core): X^T via PE-transpose, 9 QKV projections in bf16;
    K^T / V written to local DRAM and AllGathered (shared outputs); Q^T for base/ref
    stays SBUF-resident.
  B1 (query-sharded base/ref): softmax over the kv partition axis (exp -> embedded
    ones-column row sums -> reciprocal -> partition_broadcast); O^T accumulates
    SBUF-resident.
  B2 (image-row-sharded MV attention) + MV out-projection, rows AllGathered.
  C (token-sharded): base/ref out-projections + combined bias + MV rows + residual.
"""
import sys
sys.path.insert(0, '/opt/trn_rl_repo')
import contextlib
import math
import numpy as np

import concourse.bass as bass
import concourse.mybir as mybir
from concourse import bacc
from concourse.tile import TileContext
from concourse.bass_utils import run_bass_kernel_spmd
from concourse.masks import make_identity

f32 = mybir.dt.float32
bf16 = mybir.dt.bfloat16
u32 = mybir.dt.uint32
AF = mybir.ActivationFunctionType
ALU = mybir.AluOpType

NCORES = 8
BS, SEQ, C = 6, 1024, 1280
H, HD, NV = 20, 64, 6
IH = IW = 32
TOK = BS * SEQ            # 6144
TPC = TOK // NCORES       # 768
NCI = C // 128            # 10
QC = 256                  # query chunk (always batch-pure)
NQC = TPC // QC           # 3
KT = SEQ // 128           # 8
LKV = NV * IW             # 192
HDP = HD + 1              # 65: [V_h | 1] per-head stride in padded V layout
CV = H * HDP              # 1300: padded V row width
SCALE = 1.0 / math.sqrt(HD)

WNAMES = ["Wq", "Wk", "Wv", "Wq_mv", "Wk_mv", "Wv_mv", "Wq_ref", "Wk_ref",
          "Wv_ref", "Wout", "Wout_mv", "Wout_ref"]

# plan tensor layout (uint32, per core):
#   [0:12]   krow[qc*4+j]  row base (rank*C) into K{b,r}G (8C, TPC)
#   [12:24]  kcol[qc*4+j]  col base into K{b,r}G
#   [24:27]  vrow[qc]      row base (b*SEQ) into V{b,r}G (TOK, CV)
#   [27:51]  mrow[rl*6+v]  row base (rank*C) into {K,Q}mG (8C, TPC)
#   [51:75]  mcol[rl*6+v]  col base into {K,Q}mG
#   [75:79]  mvr[rl]       row base (r_gl*IW) within each SEQ block of VmG
#   [79:103] crow[tt*4+j]  row base into OmG (TOK, C)
PLAN_LEN = 128

_CACHE = {}


def _build(phases=("A", "B1", "B2", "MP", "C")):
    nc = bacc.Bacc("TRN2", target_bir_lowering=False, debug=False, num_devices=NCORES)

    hid = nc.declare_dram_parameter("hid_shard", [TPC, C], f32, isOutput=False)
    ref = nc.declare_dram_parameter("ref_shard", [TPC, C], f32, isOutput=False)
    Wn = {n: nc.declare_dram_parameter(n, [C, C], bf16, isOutput=False) for n in WNAMES}
    bsum = nc.declare_dram_parameter("bsum", [1, C], f32, isOutput=False)
    plan = nc.declare_dram_parameter("plan", [1, PLAN_LEN], u32, isOutput=False)
    out = nc.declare_dram_parameter("out_shard", [TPC, C], f32, isOutput=True)

    with TileContext(nc) as tc, contextlib.ExitStack() as stack:
        const = stack.enter_context(tc.tile_pool(name="const", bufs=1))
        resid = stack.enter_context(tc.tile_pool(name="resident", bufs=1))
        dram = stack.enter_context(tc.tile_pool(name="dram", bufs=1, space="DRAM"))

        ident = const.tile([128, 128], bf16)
        make_identity(nc, ident[:])
        ones_row = const.tile([1, 128], bf16)   # lhsT for bias broadcast
        nc.any.memset(ones_row[:], 1.0)
        bsum_bf = const.tile([1, C], bf16)
        nc.gpsimd.dma_start(bsum_bf[:], bsum[:])
        plan_sb = const.tile([1, PLAN_LEN], u32)
        nc.sync.dma_start(plan_sb[:], plan[:])

        def plan_reg(eng, idx, max_val):
            tmp = eng.alloc_register(f"plan_{idx}_{nc.next_id()}")
            eng.reg_load(tmp, plan_sb[0:1, idx:idx + 1])
            return eng.snap(tmp, donate=True, min_val=0, max_val=max_val)

        # resident bf16 tensors: 10 part-tiles of (128, TPC) each
        QbT = [resid.tile([128, TPC], bf16, name=f"QbT{i}", tag=f"QbT{i}") for i in range(NCI)]
        QrT = [resid.tile([128, TPC], bf16, name=f"QrT{i}", tag=f"QrT{i}") for i in range(NCI)]
        ObT = [resid.tile([128, TPC], bf16, name=f"ObT{i}", tag=f"ObT{i}") for i in range(NCI)]
        OrT = [resid.tile([128, TPC], bf16, name=f"OrT{i}", tag=f"OrT{i}") for i in range(NCI)]
        OmT = [resid.tile([128, TPC], bf16, name=f"OmT{i}", tag=f"OmT{i}") for i in range(NCI)]

        KbL = dram.tile([C, TPC], bf16, tag="KbL"); KrL = dram.tile([C, TPC], bf16, tag="KrL")
        KmL = dram.tile([C, TPC], bf16, tag="KmL"); QmL = dram.tile([C, TPC], bf16, tag="QmL")
        VbL = dram.tile([TPC, CV], bf16, tag="VbL"); VrL = dram.tile([TPC, CV], bf16, tag="VrL")
        VmL = dram.tile([TPC, CV], bf16, tag="VmL"); OmL = dram.tile([TPC, C], bf16, tag="OmL")
        KbG = dram.tile([NCORES * C, TPC], bf16, tag="KbG", addr_space="Shared")
        KrG = dram.tile([NCORES * C, TPC], bf16, tag="KrG", addr_space="Shared")
        VbG = dram.tile([TOK, CV], bf16, tag="VbG", addr_space="Shared")
        VrG = dram.tile([TOK, CV], bf16, tag="VrG", addr_space="Shared")
        KmG = dram.tile([NCORES * C, TPC], bf16, tag="KmG", addr_space="Shared")
        QmG = dram.tile([NCORES * C, TPC], bf16, tag="QmG", addr_space="Shared")
        VmG = dram.tile([TOK, CV], bf16, tag="VmG", addr_space="Shared")
        OmG = dram.tile([TOK, C], bf16, tag="OmG", addr_space="Shared")

        G8 = [list(range(NCORES))]

        def copyback(dst_ap, src_ap, idx):
            if idx % 2:
                nc.vector.tensor_copy(dst_ap, src_ap)
            else:
                nc.scalar.copy(dst_ap, src_ap)

        # ============================ PHASE A ============================
        with tc.tile_pool(name="pXT", bufs=1) as pXT, \
             tc.tile_pool(name="pA", bufs=2) as pA, \
             tc.tile_pool(name="pAwb", bufs=12) as pAwb, \
             tc.tile_pool(name="pAs", bufs=3) as pAs, \
             tc.tile_pool(name="psA", bufs=6, space="PSUM") as psA:

            XT = [pXT.tile([128, TPC], bf16, name=f"XT{i}", tag=f"XT{i}") for i in range(NCI)]
            RT = [pXT.tile([128, TPC], bf16, name=f"RT{i}", tag=f"RT{i}") for i in range(NCI)]
            with tc.tile_pool(name="psT", bufs=2, space="PSUM") as psT:
                for src, dstT in ((hid, XT), (ref, RT)):
                    for t in range(TPC // 128):
                        xn = pA.tile([128, C], bf16, tag="xnat")
                        nc.gpsimd.dma_start(xn[:], src[t * 128:(t + 1) * 128, :])
                        for ci in range(NCI):
                            tp = psT.tile([128, 128], bf16, tag="tp")
                            with nc.allow_low_precision(reason="bf16 transpose"):
                                nc.tensor.transpose(tp[:], xn[:, ci * 128:(ci + 1) * 128], ident[:])
                            copyback(dstT[ci][:, t * 128:(t + 1) * 128], tp[:], ci)

            def load_w_bf(wname, tag):
                tiles = []
                for ci in range(NCI):
                    wb = pAwb.tile([128, C], bf16, tag=f"wb_{tag}")
                    nc.scalar.dma_start(wb[:], Wn[wname][ci * 128:(ci + 1) * 128, :])
                    tiles.append(wb)
                return tiles

            def proj_T(wname, XTsrc, dest_sb=None, dest_dram=None):
                wt = load_w_bf(wname, "T")
                for co in range(NCI):
                    if dest_sb is not None:
                        stg = dest_sb[co]
                    else:
                        stg = pAs.tile([128, TPC], bf16, name="stgT", tag="stgT")
                    pss = [psA.tile([128, 512], f32, name=f"psT{k}", tag="psA")
                           for k in range(2)]
                    for ci in range(NCI):
                        for k in range(2):
                            nc.tensor.matmul(
                                pss[k][:, :384], wt[ci][:, co * 128:(co + 1) * 128],
                                XTsrc[ci][:, k * 384:(k + 1) * 384],
                                start=(ci == 0), stop=(ci == NCI - 1))
                    for k in range(2):
                        copyback(stg[:, k * 384:(k + 1) * 384], pss[k][:, :384], k)
                    if dest_dram is not None:
                        nc.sync.dma_start(dest_dram[co * 128:(co + 1) * 128, :], stg[:])

            def proj_V(wname, XTsrc, dest_dram):
                # out rows in padded [V_h | 1] layout: head h at cols h*65..h*65+64
                wt = load_w_bf(wname, "N")
                chunks = ((0, 512, 0, 8), (512, 1024, 8, 16), (1024, 1280, 16, 20))
                for t in range(TPC // 128):
                    stg = pAs.tile([128, CV], bf16, tag="stgV")
                    stg_h = stg[:].rearrange("p (h c) -> p h c", c=HDP)
                    nc.any.memset(stg_h[:, :, HD:HDP], 1.0)
                    pss = [psA.tile([128, 512], f32, name=f"psN{k}", tag="psA")
                           for k in range(3)]
                    for ci in range(NCI):
                        for k, (c0, c1, _, _) in enumerate(chunks):
                            nc.tensor.matmul(
                                pss[k][:, :c1 - c0], XTsrc[ci][:, t * 128:(t + 1) * 128],
                                wt[ci][:, c0:c1],
                                start=(ci == 0), stop=(ci == NCI - 1))
                    for k, (c0, c1, h0, h1) in enumerate(chunks):
                        copyback(
                            stg_h[:, h0:h1, 0:HD],
                            pss[k][:, :c1 - c0].rearrange("p (h c) -> p h c", c=HD), k)
                    nc.sync.dma_start(dest_dram[t * 128:(t + 1) * 128, :], stg[:])

            def gather(t_in, t_out):
                if "G" not in phases and "B1" not in phases:
                    return
                nc.gpsimd.collective_compute(
                    "AllGather", ALU.bypass, replica_groups=G8,
                    ins=[t_in[:].opt()], outs=[t_out[:].opt()])

            proj_T("Wk", XT, dest_dram=KbL); gather(KbL, KbG)
            proj_V("Wv", XT, VbL); gather(VbL, VbG)
            proj_T("Wk_ref", RT, dest_dram=KrL); gather(KrL, KrG)
            proj_V("Wv_ref", RT, VrL); gather(VrL, VrG)
            proj_T("Wq", XT, dest_sb=QbT)
            proj_T("Wq_ref", XT, dest_sb=QrT)
            proj_T("Wk_mv", XT, dest_dram=KmL); gather(KmL, KmG)
            proj_T("Wq_mv", XT, dest_dram=QmL); gather(QmL, QmG)
            proj_V("Wv_mv", XT, VmL); gather(VmL, VmG)

        # ============================ PHASE B1: base + ref ============================
        with tc.tile_pool(name="pB", bufs=2) as pB, \
             tc.tile_pool(name="pBk", bufs=2) as pBk, \
             tc.tile_pool(name="pBv", bufs=2) as pBv, \
             tc.tile_pool(name="psB", bufs=3, space="PSUM") as psB, \
             tc.tile_pool(name="psO", bufs=2, space="PSUM") as psO:

            KG = {"b": KbG, "r": KrG}
            VG = {"b": VbG, "r": VrG}
            QT_res = {"b": QbT, "r": QrT}
            OT_res = {"b": ObT, "r": OrT}

            for qc in range(NQC if "B1" in phases else 0):
                krows = [plan_reg(nc.sync, qc * 4 + j, 7 * C) for j in range(4)]
                kcols = [plan_reg(nc.sync, 12 + qc * 4 + j, TPC - QC) for j in range(4)]
                vrow = plan_reg(nc.sync, 24 + qc, TOK - SEQ)
                for ty in ("b", "r"):
                    # K^T for all channels: [128, ci, j, QC]; head h lives at
                    # partitions (h%2)*64.. of slab ci=h//2
                    k_sb = pBk.tile([128, NCI, 4, QC], bf16, tag="k_sb")
                    for j in range(4):
                        nc.sync.dma_start(
                            k_sb[:, :, j, :],
                            KG[ty][bass.ds(krows[j], C), bass.ds(kcols[j], QC)]
                            .rearrange("(ci p) c -> p ci c", p=128))
                    # V rows for this batch in padded [V_h | 1] layout
                    v_sb = pBv.tile([128, KT, CV], bf16, tag="v_sb")
                    nc.sync.dma_start(
                        v_sb[:],
                        VG[ty][bass.ds(vrow, SEQ), :]
                        .rearrange("(kt p) c -> p kt c", p=128))
                    for hp in range(H // 2):
                        a_sb = {}
                        for g in range(2):
                            s_ps = {}
                            for e in range(2):
                                s_ps[e] = psB.tile([128, 4, QC], f32,
                                                   name=f"s_ps{e}", tag="s_ps")
                            for kk in range(4):
                                kt = g * 4 + kk
                                for e in range(2):
                                    hb = e * 64
                                    nc.tensor.matmul(
                                        s_ps[e][:, kk, :],
                                        k_sb[hb:hb + 64, hp, kt // 2,
                                             (kt % 2) * 128:(kt % 2) * 128 + 128],
                                        QT_res[ty][hp][hb:hb + 64,
                                                       qc * QC:(qc + 1) * QC],
                                        start=True, stop=True)
                            for e in range(2):
                                ab = pB.tile([128, 4, QC], bf16,
                                             name=f"a_sb{g}{e}", tag=f"a_sb{g}{e}")
                                nc.scalar.activation(
                                    ab[:].rearrange("p a b -> p (a b)"),
                                    s_ps[e][:].rearrange("p a b -> p (a b)"),
                                    AF.Exp, scale=SCALE)
                                a_sb[(g, e)] = ab
                        for e in range(2):
                            h = hp * 2 + e
                            o_ps = psO.tile([HDP, QC], f32, tag="o_ps")
                            for kt in range(KT):
                                nc.tensor.matmul(
                                    o_ps[:], v_sb[:, kt, h * HDP:(h + 1) * HDP],
                                    a_sb[(kt // 4, e)][:, kt % 4, :],
                                    start=(kt == 0), stop=(kt == KT - 1))
                            rec = pB.tile([1, QC], f32, tag="rec")
                            nc.vector.reciprocal(rec[:], o_ps[HD:HDP, :])
                            rep = pB.tile([HD, QC], f32, tag="rep")
                            nc.gpsimd.partition_broadcast(rep[:], rec[:])
                            nc.vector.tensor_tensor(
                                out=OT_res[ty][hp][e * 64:e * 64 + 64,
                                                   qc * QC:(qc + 1) * QC],
                                in0=o_ps[0:HD, :], in1=rep[:], op=ALU.mult)

        # ============================ PHASE B2: MV attention ============================
        with tc.tile_pool(name="pM", bufs=2) as pM, \
             tc.tile_pool(name="psM", bufs=2, space="PSUM") as psM:
            VmG_v = VmG[:].rearrange("(v q) c -> v q c", q=SEQ)
            for rl in range(4 if "B2" in phases else 0):
                mrows = [plan_reg(nc.sync, 27 + rl * NV + v, 7 * C) for v in range(NV)]
                mcols = [plan_reg(nc.sync, 51 + rl * NV + v, TPC - IW) for v in range(NV)]
                mvr = plan_reg(nc.sync, 75 + rl, SEQ - IW)
                mk = pM.tile([128, NCI, NV, IW], bf16, tag="mk")
                mq = pM.tile([128, NCI, NV, IW], bf16, tag="mq")
                for tl, GT in ((mk, KmG), (mq, QmG)):
                    for v in range(NV):
                        nc.sync.dma_start(
                            tl[:, :, v, :],
                            GT[bass.ds(mrows[v], C), bass.ds(mcols[v], IW)]
                            .rearrange("(ci p) b -> p ci b", p=128))
                mv0 = pM.tile([128, CV], bf16, tag="mv0")
                nc.sync.dma_start(mv0[:], VmG_v[0:4, bass.ds(mvr, IW), :])
                mv1 = pM.tile([64, CV], bf16, tag="mv1")
                nc.sync.dma_start(mv1[:], VmG_v[4:6, bass.ds(mvr, IW), :])
                for h in range(H):
                    kv = mk[(h % 2) * 64:(h % 2) * 64 + 64, h // 2, :, :] \
                        .rearrange("p v b -> p (v b)")
                    qv = mq[(h % 2) * 64:(h % 2) * 64 + 64, h // 2, :, :] \
                        .rearrange("p v b -> p (v b)")
                    s1 = psM.tile([128, LKV], f32, tag="ms1")
                    s2 = psM.tile([64, LKV], f32, tag="ms2")
                    nc.tensor.matmul(s1[:], kv[:, 0:128], qv[:], start=True, stop=True)
                    nc.tensor.matmul(s2[:], kv[:, 128:LKV], qv[:], start=True, stop=True)
                    a1 = pM.tile([128, LKV], bf16, tag="ma1")
                    a2 = pM.tile([64, LKV], bf16, tag="ma2")
                    nc.scalar.activation(a1[:], s1[:], AF.Exp, scale=SCALE)
                    nc.scalar.activation(a2[:], s2[:], AF.Exp, scale=SCALE)
                    o_ps = psM.tile([HDP, LKV], f32, tag="mo")
                    nc.tensor.matmul(o_ps[:], mv0[:, h * HDP:(h + 1) * HDP], a1[:],
                                     start=True, stop=False)
                    nc.tensor.matmul(o_ps[:], mv1[:, h * HDP:(h + 1) * HDP], a2[:],
                                     start=False, stop=True)
                    rec = pM.tile([1, LKV], f32, tag="mrec")
                    nc.vector.reciprocal(rec[:], o_ps[HD:HDP, :])
                    rep = pM.tile([HD, LKV], f32, tag="mrep")
                    nc.gpsimd.partition_broadcast(rep[:], rec[:])
                    nc.vector.tensor_tensor(
                        out=OmT[h // 2][(h % 2) * 64:(h % 2) * 64 + 64,
                                        rl * LKV:(rl + 1) * LKV],
                        in0=o_ps[0:HD, :], in1=rep[:], op=ALU.mult)

        # MV out-projection over local rows, then gather
        with tc.tile_pool(name="pMP", bufs=3) as pMP, \
             tc.tile_pool(name="pMPb", bufs=10) as pMPb, \
             tc.tile_pool(name="psMP", bufs=4, space="PSUM") as psMP:
            wt = []
            for ci in range(NCI if "MP" in phases else 0):
                wb = pMPb.tile([128, C], bf16, tag="mw_b")
                nc.scalar.dma_start(wb[:], Wn["Wout_mv"][ci * 128:(ci + 1) * 128, :])
                wt.append(wb)
            for t in range(TPC // 128 if "MP" in phases else 0):
                stg = pMP.tile([128, C], bf16, tag="m_stg")
                for k, (c0, c1) in enumerate(((0, 512), (512, 1024), (1024, 1280))):
                    ps = psMP.tile([128, 512], f32, tag="psMP")
                    for ci in range(NCI):
                        nc.tensor.matmul(ps[:, :c1 - c0],
                                         OmT[ci][:, t * 128:(t + 1) * 128],
                                         wt[ci][:, c0:c1],
                                         start=(ci == 0), stop=(ci == NCI - 1))
                    copyback(stg[:, c0:c1], ps[:, :c1 - c0], k)
                nc.sync.dma_start(OmL[t * 128:(t + 1) * 128, :], stg[:])
            if "MP" in phases:
                nc.gpsimd.collective_compute(
                    "AllGather", ALU.bypass, replica_groups=G8,
                    ins=[OmL[:].opt()], outs=[OmG[:].opt()])

        # ============================ PHASE C ============================
        with tc.tile_pool(name="pC", bufs=2) as pC, \
             tc.tile_pool(name="pCwb", bufs=10) as pCwb, \
             tc.tile_pool(name="psC", bufs=4, space="PSUM") as psC:
            wts = {}
            for nm in (("Wout", "Wout_ref") if "C" in phases else ()):
                tl = []
                for ci in range(NCI):
                    wb = pCwb.tile([128, C], bf16, tag=f"cw_b_{nm}")
                    nc.scalar.dma_start(wb[:], Wn[nm][ci * 128:(ci + 1) * 128, :])
                    tl.append(wb)
                wts[nm] = tl
            for t in range(TPC // 128 if "C" in phases else 0):
                res_t = pC.tile([128, C], f32, tag="res")
                nc.sync.dma_start(res_t[:], hid[t * 128:(t + 1) * 128, :])
                mv_t = pC.tile([128, C], f32, tag="mvt")
                for j in range(4):
                    mo = plan_reg(nc.gpsimd, 79 + t * 4 + j, TOK - IW)
                    nc.gpsimd.dma_start(mv_t[j * IW:(j + 1) * IW, :],
                                        OmG[bass.ds(mo, IW), :])
                out_t = pC.tile([128, C], f32, tag="outt")
                chunksC = ((0, 512), (512, 1024), (1024, 1280))
                pss = [psC.tile([128, 512], f32, name=f"psC{k}", tag="psC")
                       for k in range(3)]
                first = True
                for srcT, wnm in ((ObT, "Wout"), (OrT, "Wout_ref")):
                    for ci in range(NCI):
                        for k, (c0, c1) in enumerate(chunksC):
                            nc.tensor.matmul(pss[k][:, :c1 - c0],
                                             srcT[ci][:, t * 128:(t + 1) * 128],
                                             wts[wnm][ci][:, c0:c1],
                                             start=first, stop=False)
                        first = False
                for k, (c0, c1) in enumerate(chunksC):
                    nc.tensor.matmul(pss[k][:, :c1 - c0], ones_row[:],
                                     bsum_bf[0:1, c0:c1], start=False, stop=True)
                    t1 = pC.tile([128, 512], f32, tag="t1")
                    nc.vector.tensor_tensor(out=t1[:, :c1 - c0], in0=pss[k][:, :c1 - c0],
                                            in1=res_t[:, c0:c1], op=ALU.add)
                    nc.vector.tensor_tensor(out=out_t[:, c0:c1], in0=t1[:, :c1 - c0],
                                            in1=mv_t[:, c0:c1], op=ALU.add)
                nc.sync.dma_start(out[t * 128:(t + 1) * 128, :], out_t[:])

    nc.compile()
    return nc


def _plans():
    plans = []
    for c in range(NCORES):
        p = np.zeros(PLAN_LEN, dtype=np.uint32)
        for qc in range(NQC):
            t0 = c * TPC + qc * QC
            b = t0 // SEQ
            for j in range(4):
                hcol = b * SEQ + QC * j
                rank, col = hcol // TPC, hcol % TPC
                p[qc * 4 + j] = rank * C
                p[12 + qc * 4 + j] = col
            p[24 + qc] = b * SEQ
        for rl in range(4):
            r_gl = 4 * c + rl
            for v in range(NV):
                tk = v * SEQ + IW * r_gl
                rank, col = tk // TPC, tk % TPC
                p[27 + rl * NV + v] = rank * C
                p[51 + rl * NV + v] = col
            p[75 + rl] = IW * r_gl
        for tt in range(TPC // 128):
            for j in range(4):
                t0 = c * TPC + tt * 128 + j * IW
                v, rem = divmod(t0, SEQ)
                rblk = rem // IW
                p[79 + tt * 4 + j] = (rblk // 4) * TPC + (rblk % 4) * LKV + v * IW
        plans.append(p.reshape(1, PLAN_LEN))
    return plans


def _in_maps(inputs):
    import ml_dtypes
    hid = np.asarray(inputs["hidden_states"], dtype=np.float32).reshape(TOK, C)
    ref = np.asarray(inputs["ref_hidden_states"], dtype=np.float32).reshape(TOK, C)
    bsum = (np.asarray(inputs["bout"]) + np.asarray(inputs["bout_mv"])
            + np.asarray(inputs["bout_ref"])).astype(np.float32).reshape(1, C)
    wbf = {n: np.ascontiguousarray(
        np.asarray(inputs[n], dtype=np.float32).astype(ml_dtypes.bfloat16))
        for n in WNAMES}
    plans = _plans()
    in_maps = []
    for c in range(NCORES):
        m = {
            "hid_shard": np.ascontiguousarray(hid[c * TPC:(c + 1) * TPC]),
            "ref_shard": np.ascontiguousarray(ref[c * TPC:(c + 1) * TPC]),
            "bsum": bsum,
            "plan": plans[c],
        }
        m.update(wbf)
        in_maps.append(m)
    return in_maps


def kernel(**inputs):
    if "nc" not in _CACHE:
        _CACHE["nc"] = _build()
    nc = _CACHE["nc"]
    res = run_bass_kernel_spmd(nc, _in_maps(inputs), list(range(NCORES)))
    full = np.concatenate([res.results[c]["out_shard"] for c in range(NCORES)], axis=0)
    return full.reshape(BS, SEQ, C)


if __name__ == "__main__":
    _build()
    print("BUILD OK")
